# revision 1
# baseline (speedup 1.0000x reference)
"""Trainium2 Bass kernel for single-head cross-attention.

Problem: B=16, T=L=2048, E=768 (fp32 in/out).
    Q = x @ Wq.T + bq ; K = ctx @ Wk.T + bk ; V = ctx @ Wv.T + bv
    out = softmax(Q K^T / sqrt(E)) @ V

Sharding: data-parallel over batch across 8 NeuronCores (2 batch elems per
core, weights replicated, no collectives). Per core everything is computed
with bf16 matmuls (fp32 PSUM accumulation):

  - x / context are loaded in natural [t, d] layout, cast to bf16 and
    PE-transposed into d-major chunks (matmul contracts over the partition
    dim, so both operands need d on partitions).
  - Q^T, K^T are produced in [d-chunk, t] layout (exactly what the S = Q K^T
    matmul wants); V in natural [l, e] layout (what P @ V wants).
  - S is computed per 128-query block into PSUM, softmax runs unnormalized
    (logits are bounded |s| <~ 6 for this problem, so exp never overflows and
    max-subtraction is mathematically a no-op); ScalarE's Exp activation
    produces P (bf16) and the per-row sum in one pass via accum_out.
  - P is PE-transposed, P^T @ V accumulates in PSUM, and the final rows are
    scaled by 1/rowsum while copying PSUM -> SBUF, then DMA'd out in fp32.
"""

import numpy as np
from contextlib import ExitStack

import concourse.bass as bass
import concourse.tile as tile
from concourse import bacc
from concourse import mybir
from concourse.bass_utils import run_bass_kernel_spmd
from concourse.masks import make_identity

# Problem constants (hardcoded per contract).
B, T, L, E = 16, 2048, 2048, 768
NCORES = 8
BB = B // NCORES  # batch elems per core
P = 128           # partitions
EC = E // P       # 6 chunks of the embedding dim
TSZ = 512         # t/l slice width (PSUM bank = 512 fp32)
NTS = T // TSZ    # 4
NLS = L // TSZ    # 4
NQB = T // P      # 16 query blocks per batch elem
NLC = L // P      # 16 l-chunks (P @ V contraction)
ESZ = 384         # e-slice for V / P@V (384 fp32 fits a PSUM bank)
NES = E // ESZ    # 2
SCALE = float(E) ** -0.5

F32 = mybir.dt.float32
CDT = mybir.dt.bfloat16  # matmul compute dtype


def _emit(ctx: ExitStack, tc: "tile.TileContext", x_h, c_h, w_hs, b_hs, out_h):
    nc = tc.nc
    wq_h, wk_h, wv_h = w_hs
    bq_h, bk_h, bv_h = b_hs

    const = ctx.enter_context(tc.tile_pool(name="const", bufs=1))
    big = ctx.enter_context(tc.tile_pool(name="big", bufs=1))
    loadp = ctx.enter_context(tc.tile_pool(name="loadp", bufs=8))
    castp = ctx.enter_context(tc.tile_pool(name="castp", bufs=6))
    workp = ctx.enter_context(tc.tile_pool(name="workp", bufs=2))
    attnp = ctx.enter_context(tc.tile_pool(name="attnp", bufs=2))
    psum_tp = ctx.enter_context(tc.tile_pool(name="psum_tp", bufs=3, space="PSUM"))
    psum_s = ctx.enter_context(tc.tile_pool(name="psum_s", bufs=3, space="PSUM"))
    psum_mm = ctx.enter_context(tc.tile_pool(name="psum_mm", bufs=2, space="PSUM"))

    ident = const.tile([P, P], CDT, tag="ident")
    make_identity(nc, ident)

    # ---- Weights: WT[w][dc] = W^T tile, layout [d-part(128), e-chunk(6), 128] ----
    WT = []
    with tc.tile_pool(name="wprep", bufs=6) as wprep:
        for wi, w_h in enumerate((wq_h, wk_h, wv_h)):
            rows = []
            for r in range(EC):
                wrow = wprep.tile([P, E], F32, tag="wrow")
                nc.gpsimd.dma_start(out=wrow, in_=w_h.ap()[r * P:(r + 1) * P, :])
                wrow_b = wprep.tile([P, E], CDT, tag="wrowb")
                nc.gpsimd.tensor_copy(wrow_b, wrow)
                rows.append(wrow_b)
            wts = []
            for dc in range(EC):
                pt = psum_tp.tile([P, EC, P], CDT, tag="tp")
                for r in range(EC):
                    nc.tensor.transpose(
                        pt[:, r, :], rows[r][:, dc * P:(dc + 1) * P], ident
                    )
                wt = const.tile([P, EC, P], CDT, tag=f"WT{wi}_{dc}")
                nc.vector.tensor_copy(wt, pt)
                wts.append(wt)
            WT.append(wts)

    # ---- Biases ----
    # bq/bk as per-partition scalars [128, 1] per e-chunk (Q/K live e-major).
    bqt, bkt = [], []
    for bi, (b_h, lst) in enumerate(((bq_h, bqt), (bk_h, bkt))):
        for ec2 in range(EC):
            t = const.tile([P, 1], F32, tag=f"b{bi}_{ec2}")
            nc.gpsimd.dma_start(
                out=t,
                in_=b_h.ap()[ec2 * P:(ec2 + 1) * P].rearrange("(p o) -> p o", o=1),
            )
            lst.append(t)
    # bv broadcast across partitions [128, 768] (V lives l-major).
    bvb = const.tile([P, E], F32, tag="bvb")
    bv_ap = bv_h.ap()
    nc.gpsimd.dma_start(
        out=bvb,
        in_=bass.AP(tensor=bv_ap.tensor, offset=bv_ap.offset,
                    ap=[[0, P]] + [list(a) for a in bv_ap.ap]),
    )

    for b in range(BB):
        QT = big.tile([P, EC, T], CDT, tag="QT")  # [d-part, d-chunk, t]
        KT = big.tile([P, EC, L], CDT, tag="KT")
        V = big.tile([P, NLC, E], CDT, tag="V")   # [l-part, l-chunk, e]

        # ---- projections, streamed per 512-wide slice ----
        for src in range(2):  # 0: x -> Q^T ; 1: context -> K^T and V
            src_h = x_h if src == 0 else c_h
            for ts in range(NTS):
                # Transposed source chunk [d-part, d-chunk, 4, 128] (bf16).
                sTc = workp.tile([P, EC, 4, P], CDT, tag="sTc")
                casts = []
                for j in range(4):
                    t0 = ts * TSZ + j * P
                    xl = loadp.tile([P, E], F32, tag="xload")
                    nc.gpsimd.dma_start(out=xl, in_=src_h.ap()[b, t0:t0 + P, :])
                    xc = castp.tile([P, E], CDT, tag="xcast")
                    nc.gpsimd.tensor_copy(xc, xl)
                    casts.append(xc)
                for dc in range(EC):
                    pt = psum_tp.tile([P, 4, P], CDT, tag="tp")
                    for j in range(4):
                        nc.tensor.transpose(
                            pt[:, j, :], casts[j][:, dc * P:(dc + 1) * P], ident
                        )
                    nc.vector.tensor_copy(sTc[:, dc, :, :], pt)

                if src == 0:
                    # Q^T slice: for each e-chunk accumulate over d-chunks.
                    for ec2 in range(EC):
                        mm = psum_mm.tile([P, TSZ], F32, tag="mm")
                        for dc in range(EC):
                            nc.tensor.matmul(
                                mm, lhsT=WT[0][dc][:, ec2, :],
                                rhs=sTc[:, dc, :, :],
                                start=(dc == 0), stop=(dc == EC - 1),
                            )
                        nc.scalar.activation(
                            out=QT[:, ec2, ts * TSZ:(ts + 1) * TSZ], in_=mm,
                            func=mybir.ActivationFunctionType.Identity,
                            bias=bqt[ec2], scale=1.0,
                        )
                else:
                    for ec2 in range(EC):
                        mm = psum_mm.tile([P, TSZ], F32, tag="mm")
                        for dc in range(EC):
                            nc.tensor.matmul(
                                mm, lhsT=WT[1][dc][:, ec2, :],
                                rhs=sTc[:, dc, :, :],
                                start=(dc == 0), stop=(dc == EC - 1),
                            )
                        nc.scalar.activation(
                            out=KT[:, ec2, ts * TSZ:(ts + 1) * TSZ], in_=mm,
                            func=mybir.ActivationFunctionType.Identity,
                            bias=bkt[ec2], scale=1.0,
                        )
                    # V rows for the 4 l-blocks in this slice.
                    for j in range(4):
                        lb = ts * 4 + j
                        for es in range(NES):
                            mm = psum_mm.tile([P, ESZ], F32, tag="mm")
                            for dc in range(EC):
                                nc.tensor.matmul(
                                    mm, lhsT=sTc[:, dc, j, :],
                                    rhs=WT[2][dc][:, 3 * es:3 * es + 3, :],
                                    start=(dc == 0), stop=(dc == EC - 1),
                                )
                            nc.vector.tensor_add(
                                V[:, lb, es * ESZ:(es + 1) * ESZ], mm,
                                bvb[:, es * ESZ:(es + 1) * ESZ],
                            )

        # ---- attention per 128-query block ----
        for qb in range(NQB):
            Pt = attnp.tile([P, L], CDT, tag="P")
            sums = attnp.tile([P, NLS], F32, tag="sums")
            for ls in range(NLS):
                sp = psum_s.tile([P, TSZ], F32, tag="s")
                for dc in range(EC):
                    nc.tensor.matmul(
                        sp, lhsT=QT[:, dc, qb * P:(qb + 1) * P],
                        rhs=KT[:, dc, ls * TSZ:(ls + 1) * TSZ],
                        start=(dc == 0), stop=(dc == EC - 1),
                    )
                # P = exp(S * scale); per-row partial sum via accum_out.
                nc.scalar.activation(
                    out=Pt[:, ls * TSZ:(ls + 1) * TSZ], in_=sp,
                    func=mybir.ActivationFunctionType.Exp,
                    scale=SCALE, accum_out=sums[:, ls:ls + 1],
                )
            rsum = attnp.tile([P, 1], F32, tag="rsum")
            nc.vector.reduce_sum(out=rsum, in_=sums, axis=mybir.AxisListType.X)
            recip = attnp.tile([P, 1], F32, tag="recip")
            nc.vector.reciprocal(recip, rsum)

            PT = attnp.tile([P, NLC, P], CDT, tag="PT")
            for g in range(4):
                pt = psum_tp.tile([P, 4, P], CDT, tag="tp")
                for j2 in range(4):
                    lc = g * 4 + j2
                    nc.tensor.transpose(
                        pt[:, j2, :], Pt[:, lc * P:(lc + 1) * P], ident
                    )
                nc.vector.tensor_copy(PT[:, g * 4:(g + 1) * 4, :], pt)

            outsb = attnp.tile([P, E], F32, tag="outsb")
            for es in range(NES):
                pv = psum_mm.tile([P, ESZ], F32, tag="mm")
                for lc in range(NLC):
                    nc.tensor.matmul(
                        pv, lhsT=PT[:, lc, :],
                        rhs=V[:, lc, es * ESZ:(es + 1) * ESZ],
                        start=(lc == 0), stop=(lc == NLC - 1),
                    )
                # out = (P @ V) / rowsum, fused into the PSUM -> SBUF copy.
                nc.scalar.mul(outsb[:, es * ESZ:(es + 1) * ESZ], pv, recip)
            nc.scalar.dma_start(out=out_h.ap()[b, qb * P:(qb + 1) * P, :], in_=outsb)


def build_program():
    nc = bacc.Bacc("TRN2", target_bir_lowering=False, debug=False)
    x_h = nc.dram_tensor("x", [BB, T, E], F32, kind="ExternalInput")
    c_h = nc.dram_tensor("context", [BB, L, E], F32, kind="ExternalInput")
    wq_h = nc.dram_tensor("Wq", [E, E], F32, kind="ExternalInput")
    bq_h = nc.dram_tensor("bq", [E], F32, kind="ExternalInput")
    wk_h = nc.dram_tensor("Wk", [E, E], F32, kind="ExternalInput")
    bk_h = nc.dram_tensor("bk", [E], F32, kind="ExternalInput")
    wv_h = nc.dram_tensor("Wv", [E, E], F32, kind="ExternalInput")
    bv_h = nc.dram_tensor("bv", [E], F32, kind="ExternalInput")
    out_h = nc.dram_tensor("out", [BB, T, E], F32, kind="ExternalOutput")

    with tile.TileContext(nc) as tc:
        with ExitStack() as ctx:
            _emit(ctx, tc, x_h, c_h, (wq_h, wk_h, wv_h), (bq_h, bk_h, bv_h), out_h)
    nc.compile()
    return nc


def _shard_inputs(inputs):
    arrs = {k: np.ascontiguousarray(np.asarray(v, dtype=np.float32))
            for k, v in inputs.items()}
    in_maps = []
    for c in range(NCORES):
        sl = slice(c * BB, (c + 1) * BB)
        in_maps.append({
            "x": arrs["x"][sl], "context": arrs["context"][sl],
            "Wq": arrs["Wq"], "bq": arrs["bq"],
            "Wk": arrs["Wk"], "bk": arrs["bk"],
            "Wv": arrs["Wv"], "bv": arrs["bv"],
        })
    return in_maps


def run(inputs, trace=False):
    """Build, run on 8 cores, gather. Returns (full_output, BassKernelResults)."""
    nc = build_program()
    in_maps = _shard_inputs(inputs)
    res = run_bass_kernel_spmd(nc, in_maps, core_ids=list(range(NCORES)),
                               trace=trace)
    out = np.concatenate([res.results[c]["out"] for c in range(NCORES)], axis=0)
    return out, res


def kernel(**inputs) -> np.ndarray:
    out, _ = run(inputs, trace=False)
    return out



# revision 24
# speedup vs baseline: 10.0719x; 10.0719x over previous
"""Trainium2 Bass kernel for single-head cross-attention.

Problem: B=16, T=L=2048, E=768 (fp32 in/out).
    Q = x @ Wq.T + bq ; K = ctx @ Wk.T + bk ; V = ctx @ Wv.T + bv
    out = softmax(Q K^T / sqrt(E)) @ V

Sharding: data-parallel over batch across 8 NeuronCores (2 batch elems per
core, weights replicated, no collectives). Per core, bf16 matmuls with fp32
PSUM accumulation:

  - All inputs arrive via coarse casting SWDGE DMAs (f32 DRAM -> bf16 SBUF,
    one per 512-row group / whole weight matrix) followed by one XBAR
    DMA-transpose each into d-major layout (row fold = chunk*128 + part,
    which lands exactly in [block, chunk] nested tiles). The PE array never
    runs identity-transposes and the load pipeline is ~20 DMAs deep, not 180.
  - K^T is projected e-major up front, V natural [l, e] with a constant ones
    column appended at e=768. Q is projected per 512-query group, fused into
    the attention loop (its slice feeds S^T immediately, so only one
    [128, 6, 512] Q tile lives at a time).
  - S is computed TRANSPOSED: per 128-wide l-chunk, S^T tile [l=128, t=512]
    accumulates over 6 d-chunks; Exp(S^T * scale) directly yields the P^T
    tiles that the P^T @ V matmul consumes - no P transposes. Softmax runs
    unnormalized (|logits| <~ 6 here, so exp cannot overflow).
  - P^T @ [V | 1] accumulates over 16 l-chunks; the ones column makes the
    softmax denominator fall out as output column 768. Rows are scaled by
    its reciprocal while copying PSUM -> SBUF, then DMA'd out on the
    Activation HWDGE queue (whose order matches the true dependencies).
"""

import numpy as np
from contextlib import ExitStack

import concourse.bass as bass
import concourse.tile as tile
from concourse import bacc
from concourse import mybir
from concourse.bass_utils import run_bass_kernel_spmd

# Problem constants (hardcoded per contract).
B, T, L, E = 16, 2048, 2048, 768
NCORES = 8
BB = B // NCORES  # batch elems per core
P = 128           # partitions
EC = E // P       # 6 chunks of the embedding dim
TSZ = 512         # t-slice width (PSUM bank = 512 fp32)
NTS = T // TSZ    # 4 query groups / K slices
NLC = L // P      # 16 l-chunks (S^T rows / P^T@V contraction)
GB = TSZ // P     # 4 128-row blocks per 512-row group
SCALE = float(E) ** -0.5

F32 = mybir.dt.float32
CDT = mybir.dt.bfloat16  # matmul compute dtype (fp8 was tested and rejected:
                         # quantizing any one of Q/K/P/V to e4m3 costs ~2e-2
                         # rel err on its own, right at the accuracy gate)


def _emit(ctx: ExitStack, tc: "tile.TileContext", x_h, c_h, w_hs, b_hs, out_h):
    nc = tc.nc

    const = ctx.enter_context(tc.tile_pool(name="const", bufs=1))
    big = ctx.enter_context(tc.tile_pool(name="big", bufs=1))
    loadp = ctx.enter_context(tc.tile_pool(name="loadp", bufs=3))
    qp = ctx.enter_context(tc.tile_pool(name="qp", bufs=2))
    outp = ctx.enter_context(tc.tile_pool(name="outp", bufs=3))
    attnp = ctx.enter_context(tc.tile_pool(name="attnp", bufs=4))
    psum_a = ctx.enter_context(tc.tile_pool(name="psum_a", bufs=4, space="PSUM"))
    psum_m = ctx.enter_context(tc.tile_pool(name="psum_m", bufs=4, space="PSUM"))

    # ---- Weights: Wt[w][p_d, r_e, dc, j_e] = W^T[dc*128+p_d, r_e*128+j_e].
    # One casting DMA [768,768]->bf16 [128, 6, 768], one XBAR transpose whose
    # row fold (row = c*128 + part, c = r*6+dc) lands exactly in [r, dc].
    Wt = [const.tile([P, EC, EC, P], CDT, tag=f"Wt{wi}", name=f"Wt{wi}")
          for wi in range(3)]

    def load_w(wi):
        wfull = loadp.tile([P, EC, E], CDT, tag="wfull", name="wfull")
        nc.gpsimd.dma_start(
            out=wfull, in_=w_hs[wi].ap().rearrange("(r p) d -> p r d", p=P))
        nc.sync.dma_start_transpose(out=Wt[wi], in_=wfull)

    # ---- Biases ----
    bq_h, bk_h, bv_h = b_hs
    # bq/bk as [128, 6] per-partition columns (Q/K live e-major); one DMA each.
    bq6 = const.tile([P, EC], F32, tag="bq6")
    bk6 = const.tile([P, EC], F32, tag="bk6")
    bvb = const.tile([P, E], F32, tag="bvb")

    def load_biases():
        nc.gpsimd.dma_start(out=bq6, in_=bq_h.ap().rearrange("(c p) -> p c", p=P))
        nc.gpsimd.dma_start(out=bk6, in_=bk_h.ap().rearrange("(c p) -> p c", p=P))
        # bv broadcast across partitions [128, 768] (V lives l-major).
        bv_ap = bv_h.ap()
        nc.gpsimd.dma_start(
            out=bvb,
            in_=bass.AP(tensor=bv_ap.tensor, offset=bv_ap.offset,
                        ap=[[0, P]] + [list(a) for a in bv_ap.ap]),
        )

    load_w(1)  # Wk first: K is projected first

    def load_group(src_h, b, g, dst):
        """Cast-load 512 rows [t0+j*128+p, e] and XBAR-transpose into
        dst[:, g*4:(g+1)*4, :, :] (fold c = j*6 + ec)."""
        xg = loadp.tile([P, GB, E], CDT, tag="xload", name="xg")
        nc.gpsimd.dma_start(
            out=xg,
            in_=src_h.ap()[b, g * TSZ:(g + 1) * TSZ, :].rearrange(
                "(j p) e -> p j e", p=P))
        nc.sync.dma_start_transpose(out=dst[:, g * GB:(g + 1) * GB, :, :], in_=xg)

    for b in range(BB):
        KT = big.tile([P, EC, L], CDT, tag="KT")      # [d-part, d-chunk, l]
        V = big.tile([P, NLC, E + 1], CDT, tag="V")   # [l-part, l-chunk, e | 1]
        PT = big.tile([P, NLC, TSZ], CDT, tag="PT")   # P^T tiles for one qg
        cT = big.tile([P, NLC, EC, P], CDT, tag="cT")  # ctx^T [d-part, lb, dc, j]

        # Startup-critical Pool order: Wk, ctx g0 (first K matmuls), then
        # biases + the other weights, then the remaining ctx groups.
        load_group(c_h, b, 0, cT)
        if b == 0:
            load_biases()
            load_w(2)
            load_w(0)
        for g in range(1, NTS):
            load_group(c_h, b, g, cT)
        nc.vector.memset(V[:, :, E:E + 1], 1.0)

        # ---- K projection, streamed per 512-wide l-slice ----
        for ts in range(NTS):
            for ec2 in range(EC):
                acc = psum_a.tile([P, TSZ], F32, tag="acc")
                for dc in range(EC):
                    nc.tensor.matmul(
                        acc, lhsT=Wt[1][:, ec2, dc, :],
                        rhs=cT[:, 4 * ts:4 * ts + 4, dc, :],
                        start=(dc == 0), stop=(dc == EC - 1),
                    )
                nc.scalar.activation(
                    out=KT[:, ec2, ts * TSZ:(ts + 1) * TSZ], in_=acc,
                    func=mybir.ActivationFunctionType.Identity,
                    bias=bk6[:, ec2:ec2 + 1], scale=1.0,
                )

        # ---- V projection: per 128-l block, two 384-wide halves ----
        for lb in range(NLC):
            mms = [psum_m.tile([P, TSZ], F32, tag="mm", name=f"mm{es}")
                   for es in range(2)]
            for dc in range(EC):
                for es in range(2):
                    nc.tensor.matmul(
                        mms[es][:, :384], lhsT=cT[:, lb, dc, :],
                        rhs=Wt[2][:, 3 * es:3 * es + 3, dc, :],
                        start=(dc == 0), stop=(dc == EC - 1),
                    )
            for es in range(2):
                nc.vector.tensor_add(
                    V[:, lb, es * 384:(es + 1) * 384], mms[es][:, :384],
                    bvb[:, es * 384:(es + 1) * 384],
                )

        # ---- attention per 512-query group: Q slice -> S^T -> P^T V ----
        for qg in range(NTS):
            xT = qp.tile([P, GB, EC, P], CDT, tag="xT", name="xT")
            xg = loadp.tile([P, GB, E], CDT, tag="xload", name="xg")
            nc.gpsimd.dma_start(
                out=xg,
                in_=x_h.ap()[b, qg * TSZ:(qg + 1) * TSZ, :].rearrange(
                    "(j p) e -> p j e", p=P))
            nc.sync.dma_start_transpose(out=xT, in_=xg)

            QT = qp.tile([P, EC, TSZ], CDT, tag="QT", name="QT")
            for ec2 in range(EC):
                acc = psum_a.tile([P, TSZ], F32, tag="acc")
                for dc in range(EC):
                    nc.tensor.matmul(
                        acc, lhsT=Wt[0][:, ec2, dc, :],
                        rhs=xT[:, :, dc, :],
                        start=(dc == 0), stop=(dc == EC - 1),
                    )
                nc.scalar.activation(
                    out=QT[:, ec2, :], in_=acc,
                    func=mybir.ActivationFunctionType.Identity,
                    bias=bq6[:, ec2:ec2 + 1], scale=1.0,
                )

            # S^T tiles [l=128, t=512]; Exp writes P^T directly.
            for lc in range(NLC):
                sp = psum_a.tile([P, TSZ], F32, tag="acc")
                for dc in range(EC):
                    nc.tensor.matmul(
                        sp, lhsT=KT[:, dc, lc * P:(lc + 1) * P],
                        rhs=QT[:, dc, :],
                        start=(dc == 0), stop=(dc == EC - 1),
                    )
                nc.scalar.activation(
                    out=PT[:, lc, :], in_=sp,
                    func=mybir.ActivationFunctionType.Exp, scale=SCALE,
                )

            # P^T @ [V | 1] per 128-query block; col 768 = softmax denom.
            for tsub in range(GB):
                qb = qg * GB + tsub
                pvs = [psum_m.tile([P, TSZ], F32, tag="mm", name=f"pv{es}")
                       for es in range(2)]
                for lc in range(NLC):
                    nc.tensor.matmul(
                        pvs[0], lhsT=PT[:, lc, tsub * P:(tsub + 1) * P],
                        rhs=V[:, lc, 0:TSZ],
                        start=(lc == 0), stop=(lc == NLC - 1),
                    )
                    nc.tensor.matmul(
                        pvs[1][:, :E + 1 - TSZ],
                        lhsT=PT[:, lc, tsub * P:(tsub + 1) * P],
                        rhs=V[:, lc, TSZ:E + 1],
                        start=(lc == 0), stop=(lc == NLC - 1),
                    )
                recip = attnp.tile([P, 1], F32, tag="recip")
                nc.vector.reciprocal(recip, pvs[1][:, E - TSZ:E + 1 - TSZ])
                outsb = outp.tile([P, E], F32, tag="outsb")
                nc.scalar.mul(outsb[:, 0:TSZ], pvs[0], recip)
                nc.scalar.mul(outsb[:, TSZ:E], pvs[1][:, :E - TSZ], recip)
                nc.scalar.dma_start(out=out_h.ap()[b, qb * P:(qb + 1) * P, :],
                                    in_=outsb)


def build_program():
    nc = bacc.Bacc("TRN2", target_bir_lowering=False, debug=False)
    x_h = nc.dram_tensor("x", [BB, T, E], F32, kind="ExternalInput")
    c_h = nc.dram_tensor("context", [BB, L, E], F32, kind="ExternalInput")
    wq_h = nc.dram_tensor("Wq", [E, E], F32, kind="ExternalInput")
    bq_h = nc.dram_tensor("bq", [E], F32, kind="ExternalInput")
    wk_h = nc.dram_tensor("Wk", [E, E], F32, kind="ExternalInput")
    bk_h = nc.dram_tensor("bk", [E], F32, kind="ExternalInput")
    wv_h = nc.dram_tensor("Wv", [E, E], F32, kind="ExternalInput")
    bv_h = nc.dram_tensor("bv", [E], F32, kind="ExternalInput")
    out_h = nc.dram_tensor("out", [BB, T, E], F32, kind="ExternalOutput")

    with tile.TileContext(nc) as tc:
        with ExitStack() as ctx:
            _emit(ctx, tc, x_h, c_h, (wq_h, wk_h, wv_h), (bq_h, bk_h, bv_h), out_h)
    nc.compile()
    return nc


def _shard_inputs(inputs):
    arrs = {k: np.ascontiguousarray(np.asarray(v, dtype=np.float32))
            for k, v in inputs.items()}
    in_maps = []
    for c in range(NCORES):
        sl = slice(c * BB, (c + 1) * BB)
        in_maps.append({
            "x": arrs["x"][sl], "context": arrs["context"][sl],
            "Wq": arrs["Wq"], "bq": arrs["bq"],
            "Wk": arrs["Wk"], "bk": arrs["bk"],
            "Wv": arrs["Wv"], "bv": arrs["bv"],
        })
    return in_maps


def run(inputs, trace=False):
    """Build, run on 8 cores, gather. Returns (full_output, BassKernelResults)."""
    nc = build_program()
    in_maps = _shard_inputs(inputs)
    res = run_bass_kernel_spmd(nc, in_maps, core_ids=list(range(NCORES)),
                               trace=trace)
    out = np.concatenate([res.results[c]["out"] for c in range(NCORES)], axis=0)
    return out, res


def kernel(**inputs) -> np.ndarray:
    out, _ = run(inputs, trace=False)
    return out
